# revision 5
# baseline (speedup 1.0000x reference)
"""Trainium2 Bass kernel for nn_NativeHeteroGNN (HGT-style hetero GNN).

Distribution: paper (dst) nodes are range-sharded over 8 cores (25k each);
each core processes all edges whose destination lands in its shard.
'rev' edges (paper->author) only feed the author output, which the
reference dead-code-eliminates, so they are skipped entirely.

Per core:
  - host uploads x rows (transposed, bf16) for the compact set of source
    nodes its edges touch, plus its own dst-slice x rows.
  - device builds a packed relation-transformed [kt|vt] table (bf16,
    256B rows) + a [q|xp] table for its dst slice.
  - per-edge kv rows are fetched with int32 indirect DMA gathers, q rows
    with int16 dma_gather; exp(q.k) weights and weighted values are
    scatter-added into per-bucket (128 dst nodes) PSUM accumulators via
    one-hot matmuls; softmax division + gelu + fused output linear give
    the final per-node scalar.
Host does only index manipulation / small-weight algebra; all large
tensor math runs on device.
"""

import math

import numpy as np
import ml_dtypes

import concourse.bass as bass
import concourse.mybir as mybir
from concourse import bacc
from concourse.bass_utils import run_bass_kernel_spmd
from concourse.tile import TileContext

BF16 = ml_dtypes.bfloat16
N_CORES = 8
P = 128
H, D, F, IN = 2, 32, 64, 128
N_PAPER, N_AUTHOR, E = 200000, 100000, 400000
PPC = N_PAPER // N_CORES          # papers per core
SQD = 1.0 / math.sqrt(D)

_cache = {}


def _rup(x, m):
    return (x + m - 1) // m * m


def _edge_layout(vals, ncol, dtype, fill):
    """[P, ncol] device layout with [p, i] = vals[i*128 + p]."""
    out = np.full(ncol * P, fill, dtype=dtype)
    out[: len(vals)] = vals
    return np.ascontiguousarray(out.reshape(ncol, P).T)


def _gather_idx_layout(vals, n_slots, fill):
    """int16 dma_gather idx layout: idx i at [i%16, i//16], tiled to 128 rows."""
    v = np.full(n_slots, fill, dtype=np.int16)
    v[: len(vals)] = vals
    t = v.reshape(-1, 16).T  # [16, n/16]
    return np.ascontiguousarray(np.tile(t, (8, 1)))


def _prep(inputs):
    """Host-side sharding: returns (cfg dict, per-core input maps)."""
    f32 = np.float32
    xp_ = np.asarray(inputs["x_paper"], f32)
    xa_ = np.asarray(inputs["x_author"], f32)

    # ---- small-weight algebra (host) ----
    sp = 1.0 / (1.0 + math.exp(-float(np.asarray(inputs["skip_paper"])[0])))
    aW = np.asarray(inputs["a_W_paper"], f32)
    ab = np.asarray(inputs["a_b_paper"], f32)
    oW = np.asarray(inputs["out_W"], f32)          # [64, 1]
    ob = float(np.asarray(inputs["out_b"])[0])
    W1 = (sp * aW @ oW)[:, 0]                      # [64]
    W2 = ((1.0 - sp) * oW)[:, 0]                   # [64]
    c1 = float(sp * (ab @ oW)[0] + ob)

    def blockdiag(Wk, Wv, p):
        B = np.zeros((2 * F, 2 * F), f32)
        for h in range(H):
            B[h * D:(h + 1) * D, h * D:(h + 1) * D] = Wk[h] * (float(p[h]) * SQD)
            B[F + h * D:F + (h + 1) * D, F + h * D:F + (h + 1) * D] = Wv[h]
        return B

    kv_cols = np.r_[0:F, 2 * F:3 * F]              # k cols, v cols of kqv
    tabw = {}
    for sect, (lw, lb, kw, kb, Wk, Wv, pp) in {
        "p": ("lin_W_paper", "lin_b_paper", "kqv_W_paper", "kqv_b_paper",
              "Wk_cites", "Wv_cites", "p_cites"),
        "a": ("lin_W_author", "lin_b_author", "kqv_W_author", "kqv_b_author",
              "Wk_writes", "Wv_writes", "p_writes"),
    }.items():
        linW = np.asarray(inputs[lw], f32)
        linb = np.asarray(inputs[lb], f32)
        kqvW = np.asarray(inputs[kw], f32)
        kqvb = np.asarray(inputs[kb], f32)
        B = blockdiag(np.asarray(inputs[Wk], f32), np.asarray(inputs[Wv], f32),
                      np.asarray(inputs[pp], f32))
        Wkv = kqvW[:, kv_cols] @ B                 # [64, 128]
        bkv = kqvb[kv_cols] @ B                    # [128]
        tabw[sect] = dict(
            linW=linW.astype(BF16),                # [128, 64] lhsT
            linb=linb.reshape(F, 1).astype(f32),   # [64, 1] ACT bias
            W2ext=np.vstack([Wkv, bkv[None]]).astype(BF16),  # [65, 128]
        )
    Wq = np.asarray(inputs["kqv_W_paper"], f32)[:, F:2 * F]
    bq = np.asarray(inputs["kqv_b_paper"], f32)[F:2 * F]
    Wq_ext = np.vstack([Wq, bq[None]]).astype(BF16)          # [65, 64]

    consts = dict(
        iota=np.tile(np.arange(P, dtype=f32).astype(BF16), (P, 1)),
        ident=np.eye(P, dtype=f32).astype(BF16),
        W1rep=np.tile(W1.astype(BF16), (P, 1)),
        W2rep=np.tile(W2.astype(BF16), (P, 1)),
        c1rep=np.full((P, 1), c1, f32),
        linW_p=tabw["p"]["linW"], linb_p=tabw["p"]["linb"], W2ext_p=tabw["p"]["W2ext"],
        linW_a=tabw["a"]["linW"], linb_a=tabw["a"]["linb"], W2ext_a=tabw["a"]["W2ext"],
        Wq_ext=Wq_ext,
    )

    # ---- edge sharding (host, pure index work) ----
    cw_src = np.asarray(inputs["ei_writes_src"])   # author -> paper
    cw_dst = np.asarray(inputs["ei_writes_dst"])
    cc_src = np.asarray(inputs["ei_cites_src"])    # paper -> paper
    cc_dst = np.asarray(inputs["ei_cites_dst"])

    cores = []
    for c in range(N_CORES):
        lo, hi = c * PPC, (c + 1) * PPC
        mw = (cw_dst >= lo) & (cw_dst < hi)
        mc = (cc_dst >= lo) & (cc_dst < hi)
        e_src_p = cc_src[mc]                       # paper srcs (cites)
        e_dst_p = cc_dst[mc] - lo
        e_src_a = cw_src[mw]                       # author srcs (writes)
        e_dst_a = cw_dst[mw] - lo
        up, ip = np.unique(e_src_p, return_inverse=True)
        ua, ia = np.unique(e_src_a, return_inverse=True)
        cores.append(dict(up=up, ua=ua, ip=ip, ia=ia,
                          dst=np.concatenate([e_dst_p, e_dst_a])))

    Sp = _rup(max(len(cd["up"]) for cd in cores), P)
    Sa = _rup(max(len(cd["ua"]) for cd in cores), P)
    NB = _rup(PPC, P) // P                          # buckets per core (196)

    # bucket capacity C (chunks of 128 edge slots per bucket), common to all
    maxcnt = 0
    for cd in cores:
        cnt = np.bincount(cd["dst"] // P, minlength=NB)
        maxcnt = max(maxcnt, int(cnt.max()))
    C = _rup(maxcnt, P) // P
    NCOL = NB * C

    cfg = dict(Sp=Sp, Sa=Sa, NB=NB, C=C, NCOL=NCOL, NE=NCOL * P)

    in_maps = []
    for c, cd in enumerate(cores):
        lo = c * PPC
        # compact source x, transposed, papers then authors
        xs = np.zeros((P, Sp + Sa), BF16)
        xs[:, : len(cd["up"])] = xp_[cd["up"]].T.astype(BF16)
        xs[:, Sp: Sp + len(cd["ua"])] = xa_[cd["ua"]].T.astype(BF16)
        xd = np.zeros((P, NB * P), BF16)
        xd[:, :PPC] = xp_[lo: lo + PPC].T.astype(BF16)

        # per-edge arrays in bucket-slot order
        comp = np.concatenate([cd["ip"], Sp + cd["ia"]]).astype(np.int64)
        dst = cd["dst"]
        order = np.argsort(dst // P, kind="stable")
        comp, dst = comp[order], dst[order]
        bkt = dst // P
        # slot position: base of bucket + running offset within bucket
        starts = np.zeros(NB + 1, np.int64)
        np.add.at(starts, bkt + 1, 1)
        starts = np.cumsum(starts)
        off = np.arange(len(dst)) - starts[bkt]
        slot = bkt * (C * P) + off
        NE = NCOL * P
        src_comp = np.zeros(NE, np.int32)
        src_comp[slot] = comp
        dstl = np.full(NE, 200.0, np.float32)
        dstl[slot] = (dst % P).astype(np.float32)
        qdst = np.zeros(NE, np.int32)
        qdst[slot] = dst

        m = dict(consts)
        m["x_srcT"] = xs
        m["x_dstT"] = xd
        m["src_comp"] = _edge_layout(src_comp, NCOL, np.int32, 0)
        m["dstl"] = _edge_layout(dstl.astype(BF16), NCOL, BF16, BF16(200.0))
        m["qdst"] = _edge_layout(qdst.astype(np.int32), NCOL, np.int32, 0)
        in_maps.append(m)
    return cfg, in_maps


def _build(cfg):
    Sp, Sa, NB, C, NCOL = cfg["Sp"], cfg["Sa"], cfg["NB"], cfg["C"], cfg["NCOL"]
    NE = cfg["NE"]
    bf = mybir.dt.bfloat16
    f32 = mybir.dt.float32

    nc = bacc.Bacc("TRN2", target_bir_lowering=False, debug=False,
                   num_devices=N_CORES)

    def din(name, shape, dt):
        return nc.dram_tensor(name, shape, dt, kind="ExternalInput").ap()

    x_srcT = din("x_srcT", [P, Sp + Sa], bf)
    x_dstT = din("x_dstT", [P, NB * P], bf)
    src_comp = din("src_comp", [P, NCOL], mybir.dt.int32)
    dstl = din("dstl", [P, NCOL], bf)
    qdst = din("qdst", [P, NCOL], mybir.dt.int32)
    iota = din("iota", [P, P], bf)
    ident = din("ident", [P, P], bf)
    W1rep = din("W1rep", [P, F], bf)
    W2rep = din("W2rep", [P, F], bf)
    c1rep = din("c1rep", [P, 1], f32)
    linW_p = din("linW_p", [IN, F], bf)
    linb_p = din("linb_p", [F, 1], f32)
    W2ext_p = din("W2ext_p", [F + 1, 2 * F], bf)
    linW_a = din("linW_a", [IN, F], bf)
    linb_a = din("linb_a", [F, 1], f32)
    W2ext_a = din("W2ext_a", [F + 1, 2 * F], bf)
    Wq_ext = din("Wq_ext", [F + 1, F], bf)

    out_ap = nc.dram_tensor("out", [NB * P, 1], f32, kind="ExternalOutput").ap()
    table = nc.dram_tensor("kv_table", [Sp + Sa, 2 * F], bf).ap()
    q_table = nc.dram_tensor("q_table", [NB * P, 2 * F], bf).ap()

    SLAB = 4  # node tiles per stage-1 matmul

    with TileContext(nc) as tc:
        with (
            tc.tile_pool(name="const", bufs=1) as cp,
            tc.tile_pool(name="persist", bufs=1) as pp,
        ):
            iota_t = cp.tile([P, P], bf)
            nc.sync.dma_start(out=iota_t[:], in_=iota[:])
            ident_t = cp.tile([P, P], bf)
            nc.sync.dma_start(out=ident_t[:], in_=ident[:])
            W1_t = cp.tile([P, F], bf)
            nc.sync.dma_start(out=W1_t[:], in_=W1rep[:])
            W2_t = cp.tile([P, F], bf)
            nc.sync.dma_start(out=W2_t[:], in_=W2rep[:])
            c1_t = cp.tile([P, 1], f32)
            nc.sync.dma_start(out=c1_t[:], in_=c1rep[:])
            lwp_t = cp.tile([IN, F], bf)
            nc.sync.dma_start(out=lwp_t[:], in_=linW_p[:])
            lbp_t = cp.tile([F, 1], f32)
            nc.sync.dma_start(out=lbp_t[:], in_=linb_p[:])
            w2p_t = cp.tile([F + 1, 2 * F], bf)
            nc.sync.dma_start(out=w2p_t[:], in_=W2ext_p[:])
            lwa_t = cp.tile([IN, F], bf)
            nc.sync.dma_start(out=lwa_t[:], in_=linW_a[:])
            lba_t = cp.tile([F, 1], f32)
            nc.sync.dma_start(out=lba_t[:], in_=linb_a[:])
            w2a_t = cp.tile([F + 1, 2 * F], bf)
            nc.sync.dma_start(out=w2a_t[:], in_=W2ext_a[:])
            wq_t = cp.tile([F + 1, F], bf)
            nc.sync.dma_start(out=wq_t[:], in_=Wq_ext[:])

            xp_nm = pp.tile([P, NB * F], bf)       # node-major xp per bucket
            comp_t = pp.tile([P, NCOL], mybir.dt.int32)
            nc.sync.dma_start(out=comp_t[:], in_=src_comp[:])
            dstl_t = pp.tile([P, NCOL], bf)
            nc.sync.dma_start(out=dstl_t[:], in_=dstl[:])
            qdst_t = pp.tile([P, NCOL], mybir.dt.int32)
            nc.sync.dma_start(out=qdst_t[:], in_=qdst[:])

            # ---------------- node table pass ----------------
            with (
                tc.tile_pool(name="np_sb", bufs=3) as sb,
                tc.tile_pool(name="np_ps", bufs=2, space="PSUM") as ps,
            ):
                def table_pass(base, ntile, lw_t, lb_t, w2_t):
                    for s0 in range(0, ntile, SLAB):
                        w = min(SLAB, ntile - s0) * P
                        xt = sb.tile([P, SLAB * P], bf, tag="xt")
                        nc.sync.dma_start(
                            out=xt[:, :w],
                            in_=x_srcT[:, base + s0 * P: base + s0 * P + w])
                        p1 = ps.tile([F, SLAB * P], f32, tag="p1")
                        nc.tensor.matmul(out=p1[:, :w], lhsT=lw_t[:], rhs=xt[:, :w],
                                         start=True, stop=True)
                        xr = sb.tile([F + 1, SLAB * P], bf, tag="xr")
                        nc.scalar.activation(out=xr[:F, :w], in_=p1[:, :w],
                                             func=mybir.ActivationFunctionType.Relu,
                                             bias=lb_t[:])
                        nc.vector.memset(xr[F:F + 1, :w], 1.0)
                        for i in range(w // P):
                            p2 = ps.tile([P, 2 * F], f32, tag="p2")
                            nc.tensor.matmul(out=p2[:], lhsT=xr[:, i * P:(i + 1) * P],
                                             rhs=w2_t[:], start=True, stop=True)
                            tb = sb.tile([P, 2 * F], bf, tag="tb")
                            nc.vector.tensor_copy(out=tb[:], in_=p2[:])
                            r0 = base + (s0 + i) * P
                            nc.sync.dma_start(out=table[r0: r0 + P, :], in_=tb[:])

                table_pass(0, Sp // P, lwp_t, lbp_t, w2p_t)
                table_pass(Sp, Sa // P, lwa_t, lba_t, w2a_t)

                # ---------------- q / xp pass (own dst slice) ----------------
                for s0 in range(0, NB, SLAB):
                    w = min(SLAB, NB - s0) * P
                    xt = sb.tile([P, SLAB * P], bf, tag="xt")
                    nc.sync.dma_start(out=xt[:, :w],
                                      in_=x_dstT[:, s0 * P: s0 * P + w])
                    p1 = ps.tile([F, SLAB * P], f32, tag="p1")
                    nc.tensor.matmul(out=p1[:, :w], lhsT=lwp_t[:], rhs=xt[:, :w],
                                     start=True, stop=True)
                    xr = sb.tile([F + 1, SLAB * P], bf, tag="xr")
                    nc.scalar.activation(out=xr[:F, :w], in_=p1[:, :w],
                                         func=mybir.ActivationFunctionType.Relu,
                                         bias=lbp_t[:])
                    nc.vector.memset(xr[F:F + 1, :w], 1.0)
                    for i in range(w // P):
                        t = s0 + i
                        pq = ps.tile([F, P], f32, tag="pq")
                        nc.tensor.matmul(out=pq[:], lhsT=wq_t[:],
                                         rhs=xr[:, i * P:(i + 1) * P],
                                         start=True, stop=True)
                        stk = sb.tile([P, P], bf, tag="stk")
                        nc.scalar.activation(out=stk[:F, :], in_=pq[:],
                                             func=mybir.ActivationFunctionType.Copy)
                        nc.vector.tensor_copy(out=stk[F:, :],
                                              in_=xr[:F, i * P:(i + 1) * P])
                        pt = ps.tile([P, P], bf, tag="pt")
                        nc.tensor.transpose(out=pt[:], in_=stk[:], identity=ident_t[:])
                        qrow = sb.tile([P, P], bf, tag="qrow")
                        nc.vector.tensor_copy(out=qrow[:], in_=pt[:])
                        nc.sync.dma_start(out=q_table[t * P:(t + 1) * P, :],
                                          in_=qrow[:])
                        nc.vector.tensor_copy(out=xp_nm[:, t * F:(t + 1) * F],
                                              in_=qrow[:, F:])

            # ---------------- edge pass ----------------
            BCH = 32                               # chunks per batch
            with (
                tc.tile_pool(name="eg", bufs=2) as eg,
                tc.tile_pool(name="ew", bufs=2) as ew,
                tc.tile_pool(name="eps", bufs=4, space="PSUM") as eps,
                tc.tile_pool(name="eb", bufs=3) as ebp,
            ):
                pb = None
                for col0 in range(0, NCOL, BCH):
                    nch = min(BCH, NCOL - col0)
                    ne = nch * P
                    G = eg.tile([P, BCH * P], bf, tag="G")
                    Qg = eg.tile([P, BCH * P], bf, tag="Qg")
                    for i in range(nch):
                        nc.gpsimd.indirect_dma_start(
                            out=G[:, i * P:(i + 1) * P],
                            out_offset=None,
                            in_=table[:],
                            in_offset=bass.IndirectOffsetOnAxis(
                                ap=comp_t[:, col0 + i: col0 + i + 1], axis=0),
                        )
                        nc.gpsimd.indirect_dma_start(
                            out=Qg[:, i * P:(i + 1) * P],
                            out_offset=None,
                            in_=q_table[:],
                            in_offset=bass.IndirectOffsetOnAxis(
                                ap=qdst_t[:, col0 + i: col0 + i + 1], axis=0),
                        )
                    gv = G[:, :ne].rearrange("p (a b) -> p a b", b=P)
                    qv = Qg[:, :ne].rearrange("p (a b) -> p a b", b=P)
                    prod = ew.tile([P, BCH * F], f32, tag="prod")
                    nc.vector.tensor_tensor(
                        out=prod[:, : nch * F].rearrange("p (a b) -> p a b", b=F),
                        in0=gv[:, :, :F], in1=qv[:, :, :F],
                        op=mybir.AluOpType.mult)
                    L = ew.tile([P, BCH * H], f32, tag="L")
                    nc.vector.reduce_sum(
                        out=L[:, : nch * H].rearrange("p (a b) -> p a b", b=H),
                        in_=prod[:, : nch * F].rearrange(
                            "p (a h d) -> p a h d", h=H, d=D),
                        axis=mybir.AxisListType.X)
                    RE = ew.tile([P, BCH * 66], bf, tag="RE")
                    rev = RE[:, : nch * 66].rearrange("p (a b) -> p a b", b=66)
                    nc.scalar.activation(
                        out=rev[:, :, F: F + H],
                        in_=L[:, : nch * H].rearrange("p (a b) -> p a b", b=H),
                        func=mybir.ActivationFunctionType.Exp)
                    nc.vector.tensor_tensor(
                        out=rev[:, :, :F].rearrange("p a (h d) -> p a h d", d=D),
                        in0=gv[:, :, F:].rearrange("p a (h d) -> p a h d", d=D),
                        in1=rev[:, :, F: F + H].unsqueeze(3).to_broadcast(
                            [P, nch, H, D]),
                        op=mybir.AluOpType.mult)
                    BT = ew.tile([P, BCH * P], bf, tag="BT")
                    nc.vector.tensor_tensor(
                        out=BT[:, :ne].rearrange("p (a b) -> p a b", b=P),
                        in0=dstl_t[:, col0: col0 + nch].unsqueeze(2)
                        .to_broadcast([P, nch, P]),
                        in1=iota_t[:].unsqueeze(1).to_broadcast([P, nch, P]),
                        op=mybir.AluOpType.is_equal)

                    for i in range(nch):
                        ch = col0 + i
                        ph = ch % C
                        if ph == 0:
                            pb = eps.tile([P, 66], f32, tag="pb")
                        nc.tensor.matmul(
                            out=pb[:], lhsT=BT[:, i * P:(i + 1) * P],
                            rhs=RE[:, i * 66:(i + 1) * 66],
                            start=(ph == 0), stop=(ph == C - 1))
                        if ph == C - 1:
                            b = ch // C
                            rec = ebp.tile([P, H], f32, tag="rec")
                            nc.vector.tensor_scalar_max(out=rec[:], in0=pb[:, F:],
                                                        scalar1=1e-16)
                            nc.vector.reciprocal(out=rec[:], in_=rec[:])
                            agg = ebp.tile([P, F], bf, tag="agg")
                            nc.vector.tensor_tensor(
                                out=agg[:].rearrange("p (h d) -> p h d", d=D),
                                in0=pb[:, :F].rearrange("p (h d) -> p h d", d=D),
                                in1=rec[:].unsqueeze(2).to_broadcast([P, H, D]),
                                op=mybir.AluOpType.mult)
                            gag = ebp.tile([P, F], bf, tag="gag")
                            nc.scalar.activation(
                                out=gag[:], in_=agg[:],
                                func=mybir.ActivationFunctionType.Gelu)
                            zin = ebp.tile([P, 2 * F], f32, tag="zin")
                            nc.vector.tensor_tensor(out=zin[:, :F], in0=gag[:],
                                                    in1=W1_t[:],
                                                    op=mybir.AluOpType.mult)
                            nc.vector.tensor_tensor(
                                out=zin[:, F:], in0=xp_nm[:, b * F:(b + 1) * F],
                                in1=W2_t[:], op=mybir.AluOpType.mult)
                            z0 = ebp.tile([P, 1], f32, tag="z0")
                            nc.vector.reduce_sum(out=z0[:], in_=zin[:],
                                                 axis=mybir.AxisListType.X)
                            z = ebp.tile([P, 1], f32, tag="z")
                            nc.scalar.activation(
                                out=z[:], in_=z0[:],
                                func=mybir.ActivationFunctionType.Identity,
                                bias=c1_t[:])
                            nc.sync.dma_start(out=out_ap[b * P:(b + 1) * P, :],
                                              in_=z[:])
    nc.finalize()
    return nc


def kernel(**inputs):
    cfg, in_maps = _prep(inputs)
    key = tuple(sorted(cfg.items()))
    if key not in _cache:
        _cache[key] = _build(cfg)
    nc = _cache[key]
    res = run_bass_kernel_spmd(nc, in_maps, list(range(N_CORES)))
    out = np.concatenate(
        [np.asarray(res.results[c]["out"][:PPC]) for c in range(N_CORES)], axis=0)
    return out.astype(np.float32)
